# revision 40
# baseline (speedup 1.0000x reference)
"""Trainium2 Bass kernel for nn_DecoderLayer (self-attn + cross-attn + FFN).

Sharding: 8 cores = 4 batch elements x 2 query-halves. Each core computes
its 512 query rows end-to-end (data-parallel over batch, sequence-parallel
over queries). K/V work is recomputed per core from the full per-batch
sequence -- no collectives needed.

Per-core token permutation: the core's own query rows are moved to the
front of the sequence so a single SPMD program (fixed column ranges) works
for both query-halves; the attention mask is permuted on the host to match
(softmax is invariant to key order given a consistent mask).

Attention runs in the "scores-transposed" orientation ST[k, q]:
  - ST = K @ Q^T via K=64 matmuls packed two heads per pass (row groups)
  - exp on ScalarE; softmax denominators come for free as an extra
    ones-column in the token-major V (PSUM row 64 of the AV matmul)
  - normalization is folded into the PSUM->SBUF copy using a per-head
    reciprocal row broadcast to all partitions via a K=1 ones-matmul
  - cross-attention coverage accumulates transposed (covT[k,q]) with DVE
    multiply-add passes; the host transposes it back.

All matmuls run in bf16 (fp32 accumulation in PSUM); layernorm stats,
softmax denominators, residual stream, coverage and outputs stay fp32.
"""

import numpy as np
import ml_dtypes

import concourse.bass as bass
import concourse.mybir as mybir
import concourse.tile as tile
from concourse import bacc
from concourse.bass_utils import run_bass_kernel_spmd
from concourse.masks import make_identity

# problem dims (hardcoded per the grading contract)
B, T, D, H, F = 4, 1024, 1024, 16, 4096
DH = D // H
NCORES = 8
QSH = NCORES // B          # query shards per batch element
TQ = T // QSH              # query rows per core
EPS = 1e-5
NEGMASK = -30000.0         # additive mask value (exp -> exact 0 in fp32)
P = 128
f32 = mybir.dt.float32
bf16 = mybir.dt.bfloat16
AF = mybir.ActivationFunctionType
ALU = mybir.AluOpType

NCHUNK = 512               # matmul free-dim chunk (one PSUM bank of fp32)


MAX_WAITS = 2


def _fixup_dma_waits(nc, scratch):
    """Walrus limits instructions' embedded sync commands (waits + updates
    <= 2). Tile can emit more (WAR + WAW + queue credit). Split the excess
    waits onto a chain of same-engine Memset instructions (writing a dead
    scratch byte) inserted right before -- engine program order makes this
    equivalent."""
    eng_memset = {
        mybir.EngineType.Pool: nc.gpsimd,
        mybir.EngineType.DVE: nc.vector,
    }
    for fn in nc.m.functions:
        for blk in fn.blocks:
            insts = blk.instructions
            i = 0
            while i < len(insts):
                ins = insts[i]
                so = getattr(ins, "sync_info", None)
                budget = MAX_WAITS - len(so.on_update or []) if so else 0
                budget = max(0, budget)
                if (so is not None and so.on_wait
                        and len(so.on_wait) > budget
                        and ins.engine in eng_memset):
                    waits = list(so.on_wait)
                    keep = waits[len(waits) - budget:] if budget else []
                    extra = waits[:len(waits) - budget]
                    so.on_wait = keep
                    pos = i
                    while extra:
                        chunk, extra = extra[:MAX_WAITS], extra[MAX_WAITS:]
                        nop = eng_memset[ins.engine].memset(
                            scratch[0:1, 0:1], 0.0).ins
                        nc.cur_bb.bb.instructions.remove(nop)
                        nop.sync_info = type(so)(on_wait=chunk, on_update=[])
                        insts.insert(pos, nop)
                        pos += 1
                        i += 1
                i += 1


def _build_program(t, tq, d, h, f, flags):
    """Build the single-core SPMD program."""
    dh = 64
    dtt = d // P           # feature tiles
    tt = t // P            # token tiles (kv length)
    qtt = tq // P          # own query tiles
    ft = f // P
    csd = min(NCHUNK, d)
    ncd = d // csd
    csf = min(NCHUNK, f)
    bnsub = d // min(512, d)

    # Bacc (not plain Bass): its finalize() runs generate_event_semaphores,
    # which splits multi-wait sync onto EventSemaphore carriers -- this
    # walrus build accepts at most 1 wait per instruction.
    nc = bacc.Bacc()

    # ---- DRAM I/O ----
    x_d = nc.dram_tensor("x", [t, d], f32, kind="ExternalInput")
    ctxT_d = nc.dram_tensor("ctxT", [d, t], bf16, kind="ExternalInput")
    maskT_d = nc.dram_tensor("maskT", [t, tq], f32, kind="ExternalInput")
    mcacol_d = nc.dram_tensor("mca_col", [P, tt], f32, kind="ExternalInput")
    w_d = {}
    for pre in ("sa", "ca"):
        for nm in ("q", "k", "v", "o"):
            w_d[f"{pre}_{nm}"] = nc.dram_tensor(
                f"w{nm}_{pre}", [d, d], bf16, kind="ExternalInput")
    w1_d = nc.dram_tensor("w1", [d, f], bf16, kind="ExternalInput")
    w2_d = nc.dram_tensor("w2", [f, d], bf16, kind="ExternalInput")
    bq_d = {pre: nc.dram_tensor(f"bq_{pre}", [P, dtt], f32,
                                kind="ExternalInput") for pre in ("sa", "ca")}
    bk_d = {pre: nc.dram_tensor(f"bk_{pre}", [P, dtt], f32,
                                kind="ExternalInput") for pre in ("sa", "ca")}
    b1_d = nc.dram_tensor("b1", [P, ft], f32, kind="ExternalInput")
    bv_d = {pre: nc.dram_tensor(f"bv_{pre}", [P, d], f32,
                                kind="ExternalInput") for pre in ("sa", "ca")}
    bo_d = {pre: nc.dram_tensor(f"bo_{pre}", [P, d], f32,
                                kind="ExternalInput") for pre in ("sa", "ca")}
    b2_d = nc.dram_tensor("b2", [P, d], f32, kind="ExternalInput")
    lng_d = [nc.dram_tensor(f"ln{i}_g", [P, d], f32, kind="ExternalInput")
             for i in (1, 2, 3)]
    lnb_d = [nc.dram_tensor(f"ln{i}_b", [P, d], f32, kind="ExternalInput")
             for i in (1, 2, 3)]

    xout_d = nc.dram_tensor("xout", [tq, d], f32, kind="ExternalOutput")
    covT_d = nc.dram_tensor("covT", [t, tq], f32, kind="ExternalOutput")

    with tile.TileContext(nc) as tc:
        with (
            tc.tile_pool(name="consts", bufs=1) as consts,
            tc.tile_pool(name="resid", bufs=1) as residp,
            tc.tile_pool(name="hT", bufs=1) as hTp,
            tc.tile_pool(name="kv", bufs=1) as kvp,
            tc.tile_pool(name="big", bufs=1) as bigp,
            tc.tile_pool(name="ctxp", bufs=1) as ctxp,
            tc.tile_pool(name="otn", bufs=1) as otnp,
            tc.tile_pool(name="wmat", bufs=2) as wmatp,
            tc.tile_pool(name="work", bufs=2) as work,
            tc.tile_pool(name="psA", bufs=2, space="PSUM") as psA,
            tc.tile_pool(name="psB", bufs=4, space="PSUM") as psB,
        ):
            # ---------- constants ----------
            ident = consts.tile([P, P], bf16)
            make_identity(nc, ident)
            eps_t = consts.tile([P, 1], f32)
            nc.vector.memset(eps_t, EPS)
            ones1 = consts.tile([P, P], bf16)  # row of ones at partition 64
            nc.vector.memset(ones1[64:65, :], 1.0)
            bq_sb, bk_sb = {}, {}
            for pre in ("sa", "ca"):
                bq_sb[pre] = consts.tile([P, dtt], f32, tag=f"bq{pre}",
                                         name=f"bq{pre}")
                nc.gpsimd.dma_start(out=bq_sb[pre], in_=bq_d[pre][:])
                bk_sb[pre] = consts.tile([P, dtt], f32, tag=f"bk{pre}",
                                         name=f"bk{pre}")
                nc.gpsimd.dma_start(out=bk_sb[pre], in_=bk_d[pre][:])
            b1_sb = consts.tile([P, ft], f32, tag="b1")
            nc.gpsimd.dma_start(out=b1_sb, in_=b1_d[:])

            def opt_bcast(dram, flag, tag):
                if not flag:
                    return None
                sb = consts.tile([P, d], f32, tag=tag, name=tag)
                nc.gpsimd.dma_start(out=sb, in_=dram[:])
                return sb

            bv_sb = {pre: opt_bcast(bv_d[pre], flags[f"bias_v_{pre}"],
                                    f"bv{pre}") for pre in ("sa", "ca")}
            bo_sb = {pre: opt_bcast(bo_d[pre], flags[f"bias_o_{pre}"],
                                    f"bo{pre}") for pre in ("sa", "ca")}
            b2_sb = opt_bcast(b2_d, flags["bias_2"], "b2")
            lng_sb = [opt_bcast(lng_d[i], flags[f"ln_aff{i + 1}"], f"lg{i}")
                      for i in range(3)]
            lnb_sb = [opt_bcast(lnb_d[i], flags[f"ln_aff{i + 1}"], f"lb{i}")
                      for i in range(3)]

            mca_sb = None
            if flags["mask_ca"]:
                mca_sb = consts.tile([P, tt], f32, tag="mca")
                nc.gpsimd.dma_start(out=mca_sb, in_=mcacol_d[:])

            maskT_sb = None
            if flags["mask_sa"]:
                maskT_sb = bigp.tile([P, tt, tq], f32, tag="big",
                                     name="maskT_sb")
                nc.gpsimd.dma_start(
                    out=maskT_sb,
                    in_=maskT_d[:].rearrange("(k p) q -> p k q", p=P))

            # residual stream: own tq rows, fp32 token-major
            resid = residp.tile([P, qtt, d], f32)
            nc.gpsimd.dma_start(
                out=resid, in_=x_d[0:tq, :].rearrange("(q p) d -> p q d", p=P))

            ctxT = ctxp.tile([P, dtt, t], bf16, tag="ctxT")
            nc.gpsimd.dma_start(
                out=ctxT, in_=ctxT_d[:].rearrange("(k p) t -> p k t", p=P))

            # ---------- helpers ----------
            def load_w(dram, name):
                w = wmatp.tile([P, dtt, d], bf16, tag="wmat", name=name)
                nc.gpsimd.dma_start(
                    out=w, in_=dram[:].rearrange("(k p) n -> p k n", p=P))
                return w

            def layernorm_to_T(src_ap_fn, ntiles, dst_T, iln):
                """src_ap_fn(i) -> ([P, d] f32 AP, is_dram). Writes bf16
                PE-transposed into dst_T[:, :, i*P:(i+1)*P]."""
                g_sb, b_sb = lng_sb[iln], lnb_sb[iln]
                for i in range(ntiles):
                    src, is_dram = src_ap_fn(i)
                    if is_dram:
                        xt = work.tile([P, d], f32, tag="sm4")
                        nc.gpsimd.dma_start(out=xt, in_=src)
                        src = xt
                    st = work.tile([P, bnsub, 6], f32, tag="lnst")
                    sub = d // bnsub
                    for s in range(bnsub):
                        nc.vector.bn_stats(
                            out=st[:, s, :],
                            in_=src[:, s * sub:(s + 1) * sub])
                    mv = work.tile([P, 2], f32, tag="lnmv")
                    nc.vector.bn_aggr(out=mv, in_=st)
                    rstd = work.tile([P, 1], f32, tag="lnrs")
                    nc.scalar.activation(out=rstd, in_=mv[:, 1:2],
                                         func=AF.Sqrt, bias=eps_t, scale=1.0)
                    nc.vector.reciprocal(out=rstd, in_=rstd)
                    nmr = work.tile([P, 1], f32, tag="lnnm")
                    nc.vector.tensor_mul(out=nmr, in0=mv[:, 0:1], in1=rstd)
                    nc.vector.tensor_scalar_mul(out=nmr, in0=nmr, scalar1=-1.0)
                    hb = work.tile([P, d], bf16, tag="lnh")
                    if g_sb is None:
                        nc.vector.tensor_scalar(
                            out=hb, in0=src, scalar1=rstd, scalar2=nmr,
                            op0=ALU.mult, op1=ALU.add)
                    else:
                        hf = work.tile([P, d], f32, tag="lnhf")
                        nc.vector.tensor_scalar(
                            out=hf, in0=src, scalar1=rstd, scalar2=nmr,
                            op0=ALU.mult, op1=ALU.add)
                        nc.vector.tensor_mul(out=hf, in0=hf, in1=g_sb)
                        nc.vector.tensor_add(out=hb, in0=hf, in1=b_sb)
                    for dt_i in range(dtt):
                        tp = psB.tile([P, P], bf16, tag="psB", name="tp")
                        nc.tensor.transpose(tp,
                                            hb[:, dt_i * P:(dt_i + 1) * P],
                                            ident)
                        nc.vector.tensor_copy(
                            out=dst_T[:, dt_i, i * P:(i + 1) * P],
                            in_=tp)

            def proj_fm(dst_T, srcT, w_sb, ncols, b_sb, scale):
                """Feature-major projection: dst_T [P, dtt, ncols] bf16."""
                nchunks = max(1, ncols // NCHUNK)
                cs = min(NCHUNK, ncols)
                for m in range(dtt):
                    for ch in range(nchunks):
                        ps = psB.tile([P, cs], f32, tag="psB", name="pj")
                        for k in range(dtt):
                            nc.tensor.matmul(
                                ps, w_sb[:, k, m * P:(m + 1) * P],
                                srcT[:, k, ch * cs:(ch + 1) * cs],
                                start=(k == 0), stop=(k == dtt - 1))
                        nc.vector.tensor_scalar(
                            out=dst_T[:, m, ch * cs:(ch + 1) * cs], in0=ps,
                            scalar1=scale, scalar2=b_sb[:, m:m + 1],
                            op0=ALU.mult, op1=ALU.add)

            def proj_vaug(dst, srcT, w_sb, b_bcast):
                """Token-major V with per-head ones column:
                dst [P, tt, h*65] bf16; head hh at cols hh*65..hh*65+64,
                col hh*65+64 == 1.0 (softmax denominator trick)."""
                for ti in range(tt):
                    for ch in range(ncd):
                        ps = psB.tile([P, csd], f32, tag="psB", name="pv")
                        for k in range(dtt):
                            nc.tensor.matmul(
                                ps, srcT[:, k, ti * P:(ti + 1) * P],
                                w_sb[:, k, ch * csd:(ch + 1) * csd],
                                start=(k == 0), stop=(k == dtt - 1))
                        hperc = csd // dh    # heads per chunk
                        for hj in range(hperc):
                            hh = ch * hperc + hj
                            o = dst[:, ti, hh * (dh + 1):hh * (dh + 1) + dh]
                            if b_bcast is None:
                                nc.vector.tensor_copy(
                                    out=o, in_=ps[:, hj * dh:(hj + 1) * dh])
                            else:
                                nc.vector.tensor_add(
                                    out=o, in0=ps[:, hj * dh:(hj + 1) * dh],
                                    in1=b_bcast[:, ch * csd + hj * dh:
                                                ch * csd + (hj + 1) * dh])
                    ones_ap = dst[:, ti, :].rearrange(
                        "p (hh c) -> p hh c", c=dh + 1)[:, :, dh:dh + 1]
                    nc.vector.memset(ones_ap, 1.0)

            def attention(qT, kT, vaug, use_maskT, use_mca, wo_sb, bo_bcast,
                          do_cov, covT_sb):
                OTn = otnp.tile([P, dtt, tq], bf16, tag="OTn", name="OTn")
                if do_cov:
                    nc.vector.memset(covT_sb, 0.0)
                for hh in range(h):
                    pr, sub = hh // 2, hh % 2
                    lo = 64 * sub
                    # ST scores + exp, per k-tile
                    est = work.tile([P, tt, tq], bf16, tag="est", name="est")
                    for kt in range(tt):
                        sp = psB.tile([P, tq], f32, tag="psB", name="sps")
                        nc.tensor.matmul(
                            sp[:, 0:tq],
                            kT[lo:lo + dh, pr, kt * P:(kt + 1) * P],
                            qT[lo:lo + dh, pr, :],
                            start=True, stop=True, tile_position=(lo, 0))
                        if use_maskT:
                            sm = work.tile([P, tq], f32, tag="sm4",
                                           name="sm")
                            nc.vector.tensor_add(out=sm, in0=sp,
                                                 in1=maskT_sb[:, kt, :])
                            nc.scalar.activation(out=est[:, kt, :], in_=sm,
                                                 func=AF.Exp, bias=0.0,
                                                 scale=1.0)
                        elif use_mca:
                            nc.scalar.activation(
                                out=est[:, kt, :], in_=sp, func=AF.Exp,
                                bias=mca_sb[:, kt:kt + 1], scale=1.0)
                        else:
                            nc.scalar.activation(out=est[:, kt, :], in_=sp,
                                                 func=AF.Exp, bias=0.0,
                                                 scale=1.0)
                    # AV with fused denominator (row 64)
                    av = psB.tile([P, tq], f32, tag="psB", name="av")
                    for kt in range(tt):
                        nc.tensor.matmul(
                            av[0:dh + 1, :],
                            vaug[:, kt, hh * (dh + 1):(hh + 1) * (dh + 1)],
                            est[:, kt, :],
                            start=(kt == 0), stop=(kt == tt - 1))
                    # reciprocal of denominators (partition 64), broadcast
                    # to all partitions via K=1 ones-matmul
                    rb_sb = work.tile([P, tq], bf16, tag="rb", name="rb")
                    with nc.allow_low_precision(
                            reason="softmax recip row in bf16, matches "
                                   "bf16 probability envelope"):
                        nc.vector.reciprocal(out=rb_sb[64:65, :],
                                             in_=av[64:65, :])
                    rb_ps = psB.tile([P, tq], f32, tag="psB", name="rbps")
                    nc.tensor.matmul(rb_ps, ones1[64:65, :],
                                     rb_sb[64:65, :], start=True,
                                     stop=True, tile_position=(64, 0))
                    nc.vector.tensor_copy(out=rb_sb, in_=rb_ps)
                    # normalized output rows for this head (feature-major).
                    # DVE cannot shift partitions, so odd heads (rows 64:128
                    # of OTn) go through a small partition-shifting DMA.
                    if lo == 0:
                        nc.vector.tensor_mul(
                            out=OTn[0:dh, pr, :], in0=av[0:dh, :],
                            in1=rb_sb[0:dh, :])
                    else:
                        otmp = work.tile([P, tq], bf16, tag="otmp",
                                         name="otmp")
                        nc.vector.tensor_mul(
                            out=otmp[0:dh, :], in0=av[0:dh, :],
                            in1=rb_sb[0:dh, :])
                        nc.gpsimd.dma_start(out=OTn[lo:lo + dh, pr, :],
                                          in_=otmp[0:dh, :])
                    if do_cov:
                        # covT += est * recip  (fp32, per k-tile)
                        for kt in range(tt):
                            pt = work.tile([P, tq], f32, tag="sm4",
                                           name="pt")
                            nc.vector.tensor_mul(out=pt, in0=est[:, kt, :],
                                                 in1=rb_sb)
                            nc.vector.tensor_add(out=covT_sb[:, kt, :],
                                                 in0=covT_sb[:, kt, :],
                                                 in1=pt)
                # output projection + residual, per query tile
                for qt in range(qtt):
                    op = psA.tile([P, d], f32, tag="psA", name="op")
                    for ch in range(ncd):
                        for k in range(dtt):
                            nc.tensor.matmul(
                                op[:, ch * csd:(ch + 1) * csd],
                                OTn[:, k, qt * P:(qt + 1) * P],
                                wo_sb[:, k, ch * csd:(ch + 1) * csd],
                                start=(k == 0), stop=(k == dtt - 1))
                    if bo_bcast is not None:
                        nc.vector.tensor_add(out=op, in0=op, in1=bo_bcast)
                    nc.vector.tensor_add(out=resid[:, qt, :],
                                         in0=resid[:, qt, :], in1=op)
                if do_cov:
                    for kt in range(tt):
                        cs_t = work.tile([P, tq], f32, tag="sm4", name="cs")
                        nc.vector.tensor_scalar_mul(out=cs_t,
                                                    in0=covT_sb[:, kt, :],
                                                    scalar1=1.0 / h)
                        nc.gpsimd.dma_start(
                            out=covT_d[kt * P:(kt + 1) * P, :], in_=cs_t)

            # ---------- phase 1: LN1 over full sequence -> h1T ----------
            h1T = hTp.tile([P, dtt, t], bf16, tag="hT", name="h1T")
            layernorm_to_T(
                lambda i: (x_d[i * P:(i + 1) * P, :], True), tt, h1T, 0)

            # ---------- phase 2: self-attention ----------
            wq = load_w(w_d["sa_q"], "wqs")
            qT = kvp.tile([P, dtt, tq], bf16, tag="qT", name="qT1")
            proj_fm(qT, h1T[:, :, 0:tq], wq, tq, bq_sb["sa"], dh ** -0.5)
            wk = load_w(w_d["sa_k"], "wks")
            kT = kvp.tile([P, dtt, t], bf16, tag="kT", name="kT1")
            proj_fm(kT, h1T, wk, t, bk_sb["sa"], 1.0)
            wv = load_w(w_d["sa_v"], "wvs")
            vaug = kvp.tile([P, tt, h * (dh + 1)], bf16, tag="vaug",
                            name="va1")
            proj_vaug(vaug, h1T, wv, bv_sb["sa"])
            wo = load_w(w_d["sa_o"], "wos")
            attention(qT, kT, vaug, flags["mask_sa"], False, wo, bo_sb["sa"],
                      False, None)

            # ---------- phase 3: LN2 -> h2T, cross-attention ----------
            h2T = hTp.tile([P, dtt, tq], bf16, tag="hT", name="h2T")
            layernorm_to_T(lambda i: (resid[:, i, :], False), qtt, h2T, 1)
            wq = load_w(w_d["ca_q"], "wqc")
            qT2 = kvp.tile([P, dtt, tq], bf16, tag="qT", name="qT2")
            proj_fm(qT2, h2T, wq, tq, bq_sb["ca"], dh ** -0.5)
            wk = load_w(w_d["ca_k"], "wkc")
            kT2 = kvp.tile([P, dtt, t], bf16, tag="kT", name="kT2")
            proj_fm(kT2, ctxT, wk, t, bk_sb["ca"], 1.0)
            wv = load_w(w_d["ca_v"], "wvc")
            vaug2 = kvp.tile([P, tt, h * (dh + 1)], bf16, tag="vaug",
                             name="va2")
            proj_vaug(vaug2, ctxT, wv, bv_sb["ca"])
            wo = load_w(w_d["ca_o"], "woc")
            covT_sb = bigp.tile([P, tt, tq], f32, tag="big", name="covT_sb")
            attention(qT2, kT2, vaug2, False, flags["mask_ca"], wo,
                      bo_sb["ca"], True, covT_sb)

            # ---------- phase 4: LN3 -> h3T, FFN ----------
            h3T = hTp.tile([P, dtt, tq], bf16, tag="hT", name="h3T")
            layernorm_to_T(lambda i: (resid[:, i, :], False), qtt, h3T, 2)
            fT = bigp.tile([P, ft, tq], bf16, tag="big", name="fT")
            for fc in range(f // csf):
                w1c = wmatp.tile([P, dtt, csf], bf16, tag="wmat", name="w1c")
                nc.gpsimd.dma_start(
                    out=w1c,
                    in_=w1_d[:, fc * csf:(fc + 1) * csf].rearrange(
                        "(k p) n -> p k n", p=P))
                for fm in range(csf // P):
                    fi = fc * (csf // P) + fm
                    ps = psB.tile([P, tq], f32, tag="psB", name="pf")
                    for k in range(dtt):
                        nc.tensor.matmul(
                            ps, w1c[:, k, fm * P:(fm + 1) * P],
                            h3T[:, k, :],
                            start=(k == 0), stop=(k == dtt - 1))
                    nc.scalar.activation(out=fT[:, fi, :], in_=ps,
                                         func=AF.Relu,
                                         bias=b1_sb[:, fi:fi + 1], scale=1.0)
            # FF2: stream W2 in dtt-row chunks; two qt outputs per sweep
            kchunks = ft // dtt
            for half in range((qtt + 1) // 2):
                qts = [q for q in (2 * half, 2 * half + 1) if q < qtt]
                ops = [psA.tile([P, d], f32, tag="psA", name=f"op{qi}")
                       for qi in range(len(qts))]
                for kc in range(kchunks):
                    w2c = wmatp.tile([P, dtt, d], bf16, tag="wmat",
                                     name="w2c")
                    nc.gpsimd.dma_start(
                        out=w2c,
                        in_=w2_d[kc * dtt * P:(kc + 1) * dtt * P, :].rearrange(
                            "(k p) n -> p k n", p=P))
                    for qi, qt in enumerate(qts):
                        for ch in range(ncd):
                            for k in range(dtt):
                                kk = kc * dtt + k
                                nc.tensor.matmul(
                                    ops[qi][:, ch * csd:(ch + 1) * csd],
                                    fT[:, kk, qt * P:(qt + 1) * P],
                                    w2c[:, k, ch * csd:(ch + 1) * csd],
                                    start=(kk == 0), stop=(kk == ft - 1))
                for qi, qt in enumerate(qts):
                    if b2_sb is not None:
                        nc.vector.tensor_add(out=ops[qi], in0=ops[qi],
                                             in1=b2_sb)
                    nc.vector.tensor_add(out=resid[:, qt, :],
                                         in0=resid[:, qt, :], in1=ops[qi])
                    nc.gpsimd.dma_start(out=xout_d[qt * P:(qt + 1) * P, :],
                                      in_=resid[:, qt, :])
    nc.finalize()   # Bacc legalization (reg alloc, event-semaphore splits)
    return nc


def _prep_inputs(x, context, mask_tgt, mask_src, weights, t, tq, d, h, f):
    """Build per-core in_maps. Returns (in_maps, flags, perms)."""
    ft, dtt, tt = f // P, d // P, t // P
    b = x.shape[0]
    qsh = t // tq

    def bf(a):
        return np.ascontiguousarray(a.astype(ml_dtypes.bfloat16))

    def f32c(a):
        return np.ascontiguousarray(a.astype(np.float32))

    def pp(vec, ntiles, scale=1.0):
        return f32c((vec.astype(np.float32) * scale).reshape(ntiles, P).T)

    def bcast(vec):
        return f32c(np.broadcast_to(vec.astype(np.float32)[None, :], (P, d)))

    flags = {
        "mask_sa": bool(mask_tgt.any()),
        "mask_ca": bool(mask_src.any()),
        "bias_v_sa": bool(np.any(weights["sa_bv"])),
        "bias_v_ca": bool(np.any(weights["ca_bv"])),
        "bias_o_sa": bool(np.any(weights["sa_bo"])),
        "bias_o_ca": bool(np.any(weights["ca_bo"])),
        "bias_2": bool(np.any(weights["ff_b2"])),
        "ln_aff1": not (np.all(weights["ln1_g"] == 1)
                        and not np.any(weights["ln1_b"])),
        "ln_aff2": not (np.all(weights["ln2_g"] == 1)
                        and not np.any(weights["ln2_b"])),
        "ln_aff3": not (np.all(weights["ln3_g"] == 1)
                        and not np.any(weights["ln3_b"])),
    }

    shared = {
        "wq_sa": bf(weights["sa_Wq"]), "wk_sa": bf(weights["sa_Wk"]),
        "wv_sa": bf(weights["sa_Wv"]), "wo_sa": bf(weights["sa_Wo"]),
        "wq_ca": bf(weights["ca_Wq"]), "wk_ca": bf(weights["ca_Wk"]),
        "wv_ca": bf(weights["ca_Wv"]), "wo_ca": bf(weights["ca_Wo"]),
        "w1": bf(weights["ff_W1"]), "w2": bf(weights["ff_W2"]),
        "bq_sa": pp(weights["sa_bq"], dtt, (d // h) ** -0.5),
        "bq_ca": pp(weights["ca_bq"], dtt, (d // h) ** -0.5),
        "bk_sa": pp(weights["sa_bk"], dtt),
        "bk_ca": pp(weights["ca_bk"], dtt),
        "b1": pp(weights["ff_b1"], ft),
        "bv_sa": bcast(weights["sa_bv"]), "bv_ca": bcast(weights["ca_bv"]),
        "bo_sa": bcast(weights["sa_bo"]), "bo_ca": bcast(weights["ca_bo"]),
        "b2": bcast(weights["ff_b2"]),
    }
    for nm in ("ln1", "ln2", "ln3"):
        shared[f"{nm}_g"] = bcast(weights[f"{nm}_g"])
        shared[f"{nm}_b"] = bcast(weights[f"{nm}_b"])

    mt = np.broadcast_to(mask_tgt, (b, t, t))
    ms = np.broadcast_to(mask_src.reshape(b, -1), (b, t))

    in_maps, perms = [], []
    for c in range(b * qsh):
        bi, qh = c // qsh, c % qsh
        qs = qh * tq
        perm = np.concatenate([np.arange(qs, qs + tq),
                               np.arange(0, qs),
                               np.arange(qs + tq, t)]).astype(np.int64)
        perms.append(perm)
        # maskT[k, q]: keys in permuted order, queries = own rows
        m = (mt[bi][qs:qs + tq][:, perm]).astype(np.float32).T * NEGMASK
        # mask_ca as per-k column bias [P, tt] (k = kt*P + p)
        mca = (ms[bi].astype(np.float32) * NEGMASK).reshape(tt, P).T
        im = dict(shared)
        im["x"] = f32c(x[bi][perm])
        im["ctxT"] = bf(context[bi].T)
        im["maskT"] = f32c(m)
        im["mca_col"] = f32c(mca)
        in_maps.append(im)
    return in_maps, flags, perms


_CACHE = {}


def _get_program(key, t, tq, d, h, f, flags):
    ck = (key, tuple(sorted(flags.items())))
    if ck not in _CACHE:
        _CACHE[ck] = _build_program(t, tq, d, h, f, flags)
    return _CACHE[ck]


def kernel(x, context, mask_tgt, mask_src,
           ln1_g, ln1_b, ln2_g, ln2_b, ln3_g, ln3_b,
           sa_Wq, sa_bq, sa_Wk, sa_bk, sa_Wv, sa_bv, sa_Wo, sa_bo,
           ca_Wq, ca_bq, ca_Wk, ca_bk, ca_Wv, ca_bv, ca_Wo, ca_bo,
           ff_W1, ff_b1, ff_W2, ff_b2, _run=None):
    x = np.asarray(x, np.float32)
    context = np.asarray(context, np.float32)
    mask_tgt = np.asarray(mask_tgt, bool)
    mask_src = np.asarray(mask_src, bool)
    weights = {k: np.asarray(v) for k, v in dict(
        ln1_g=ln1_g, ln1_b=ln1_b, ln2_g=ln2_g, ln2_b=ln2_b,
        ln3_g=ln3_g, ln3_b=ln3_b,
        sa_Wq=sa_Wq, sa_bq=sa_bq, sa_Wk=sa_Wk, sa_bk=sa_bk,
        sa_Wv=sa_Wv, sa_bv=sa_bv, sa_Wo=sa_Wo, sa_bo=sa_bo,
        ca_Wq=ca_Wq, ca_bq=ca_bq, ca_Wk=ca_Wk, ca_bk=ca_bk,
        ca_Wv=ca_Wv, ca_bv=ca_bv, ca_Wo=ca_Wo, ca_bo=ca_bo,
        ff_W1=ff_W1, ff_b1=ff_b1, ff_W2=ff_W2, ff_b2=ff_b2).items()}

    t, tq, d, h, f = T, TQ, D, H, F
    in_maps, flags, perms = _prep_inputs(
        x, context, mask_tgt, mask_src, weights, t, tq, d, h, f)
    nc = _get_program("full", t, tq, d, h, f, flags)
    if _run is None:
        res = run_bass_kernel_spmd(nc, in_maps,
                                   core_ids=list(range(len(in_maps)))).results
    else:
        res = _run(nc, in_maps)

    b, qsh = x.shape[0], t // tq
    xout = np.empty((b, t, d), np.float32)
    cov = np.empty((b, t, t), np.float32)
    for c, r in enumerate(res):
        bi, qh = c // qsh, c % qsh
        qs = qh * tq
        xout[bi, qs:qs + tq] = r["xout"]
        cov[bi, qs:qs + tq] = r["covT"].T
    return xout, cov


# revision 41
# speedup vs baseline: 40.2920x; 40.2920x over previous
"""Trainium2 Bass kernel for nn_DecoderLayer (self-attn + cross-attn + FFN).

Sharding: 8 cores = 4 batch elements x 2 query-halves. Each core computes
its 512 query rows end-to-end (data-parallel over batch, sequence-parallel
over queries). K/V work is recomputed per core from the full per-batch
sequence -- no collectives needed.

Per-core token permutation: the core's own query rows are moved to the
front of the sequence so a single SPMD program (fixed column ranges) works
for both query-halves; the attention mask is permuted on the host to match
(softmax is invariant to key order given a consistent mask).

Attention runs in the "scores-transposed" orientation ST[k, q]:
  - ST = K @ Q^T via K=64 matmuls packed two heads per pass (row groups)
  - exp on ScalarE; softmax denominators come for free as an extra
    ones-column in the token-major V (PSUM row 64 of the AV matmul)
  - normalization is folded into the PSUM->SBUF copy using a per-head
    reciprocal row broadcast to all partitions via a K=1 ones-matmul
  - cross-attention coverage accumulates transposed (covT[k,q]) with DVE
    multiply-add passes; the host transposes it back.

All matmuls run in bf16 (fp32 accumulation in PSUM); layernorm stats,
softmax denominators, residual stream, coverage and outputs stay fp32.
"""

import numpy as np
import ml_dtypes

import concourse.bass as bass
import concourse.mybir as mybir
import concourse.tile as tile
from concourse import bacc
from concourse.bass_utils import run_bass_kernel_spmd
from concourse.masks import make_identity

# problem dims (hardcoded per the grading contract)
B, T, D, H, F = 4, 1024, 1024, 16, 4096
DH = D // H
NCORES = 8
QSH = NCORES // B          # query shards per batch element
TQ = T // QSH              # query rows per core
EPS = 1e-5
NEGMASK = -30000.0         # additive mask value (exp -> exact 0 in fp32)
P = 128
f32 = mybir.dt.float32
bf16 = mybir.dt.bfloat16
AF = mybir.ActivationFunctionType
ALU = mybir.AluOpType

NCHUNK = 512               # matmul free-dim chunk (one PSUM bank of fp32)


MAX_WAITS = 2


def _fixup_dma_waits(nc, scratch):
    """Walrus limits instructions' embedded sync commands (waits + updates
    <= 2). Tile can emit more (WAR + WAW + queue credit). Split the excess
    waits onto a chain of same-engine Memset instructions (writing a dead
    scratch byte) inserted right before -- engine program order makes this
    equivalent."""
    eng_memset = {
        mybir.EngineType.Pool: nc.gpsimd,
        mybir.EngineType.DVE: nc.vector,
    }
    for fn in nc.m.functions:
        for blk in fn.blocks:
            insts = blk.instructions
            i = 0
            while i < len(insts):
                ins = insts[i]
                so = getattr(ins, "sync_info", None)
                budget = MAX_WAITS - len(so.on_update or []) if so else 0
                budget = max(0, budget)
                if (so is not None and so.on_wait
                        and len(so.on_wait) > budget
                        and ins.engine in eng_memset):
                    waits = list(so.on_wait)
                    keep = waits[len(waits) - budget:] if budget else []
                    extra = waits[:len(waits) - budget]
                    so.on_wait = keep
                    pos = i
                    while extra:
                        chunk, extra = extra[:MAX_WAITS], extra[MAX_WAITS:]
                        nop = eng_memset[ins.engine].memset(
                            scratch[0:1, 0:1], 0.0).ins
                        nc.cur_bb.bb.instructions.remove(nop)
                        nop.sync_info = type(so)(on_wait=chunk, on_update=[])
                        insts.insert(pos, nop)
                        pos += 1
                        i += 1
                i += 1


def _build_program(t, tq, d, h, f, flags):
    """Build the single-core SPMD program."""
    dh = 64
    dtt = d // P           # feature tiles
    tt = t // P            # token tiles (kv length)
    qtt = tq // P          # own query tiles
    ft = f // P
    csd = min(NCHUNK, d)
    ncd = d // csd
    csf = min(NCHUNK, f)
    bnsub = d // min(512, d)

    # Bacc (not plain Bass): its finalize() runs generate_event_semaphores,
    # which splits multi-wait sync onto EventSemaphore carriers -- this
    # walrus build accepts at most 1 wait per instruction.
    nc = bacc.Bacc()

    # ---- DRAM I/O ----
    x_d = nc.dram_tensor("x", [t, d], f32, kind="ExternalInput")
    ctxT_d = nc.dram_tensor("ctxT", [d, t], bf16, kind="ExternalInput")
    maskT_d = nc.dram_tensor("maskT", [t, tq], f32, kind="ExternalInput")
    mcacol_d = nc.dram_tensor("mca_col", [P, tt], f32, kind="ExternalInput")
    w_d = {}
    for pre in ("sa", "ca"):
        for nm in ("q", "k", "v", "o"):
            w_d[f"{pre}_{nm}"] = nc.dram_tensor(
                f"w{nm}_{pre}", [d, d], bf16, kind="ExternalInput")
    w1_d = nc.dram_tensor("w1", [d, f], bf16, kind="ExternalInput")
    w2_d = nc.dram_tensor("w2", [f, d], bf16, kind="ExternalInput")
    bq_d = {pre: nc.dram_tensor(f"bq_{pre}", [P, dtt], f32,
                                kind="ExternalInput") for pre in ("sa", "ca")}
    bk_d = {pre: nc.dram_tensor(f"bk_{pre}", [P, dtt], f32,
                                kind="ExternalInput") for pre in ("sa", "ca")}
    b1_d = nc.dram_tensor("b1", [P, ft], f32, kind="ExternalInput")
    bv_d = {pre: nc.dram_tensor(f"bv_{pre}", [P, d], f32,
                                kind="ExternalInput") for pre in ("sa", "ca")}
    bo_d = {pre: nc.dram_tensor(f"bo_{pre}", [P, d], f32,
                                kind="ExternalInput") for pre in ("sa", "ca")}
    b2_d = nc.dram_tensor("b2", [P, d], f32, kind="ExternalInput")
    lng_d = [nc.dram_tensor(f"ln{i}_g", [P, d], f32, kind="ExternalInput")
             for i in (1, 2, 3)]
    lnb_d = [nc.dram_tensor(f"ln{i}_b", [P, d], f32, kind="ExternalInput")
             for i in (1, 2, 3)]

    xout_d = nc.dram_tensor("xout", [tq, d], f32, kind="ExternalOutput")
    covT_d = nc.dram_tensor("covT", [t, tq], f32, kind="ExternalOutput")

    with tile.TileContext(nc) as tc:
        with (
            tc.tile_pool(name="consts", bufs=1) as consts,
            tc.tile_pool(name="resid", bufs=1) as residp,
            tc.tile_pool(name="hT", bufs=1) as hTp,
            tc.tile_pool(name="kv", bufs=1) as kvp,
            tc.tile_pool(name="big", bufs=1) as bigp,
            tc.tile_pool(name="ctxp", bufs=1) as ctxp,
            tc.tile_pool(name="otn", bufs=1) as otnp,
            tc.tile_pool(name="wmat", bufs=2) as wmatp,
            tc.tile_pool(name="work", bufs=2) as work,
            tc.tile_pool(name="psA", bufs=2, space="PSUM") as psA,
            tc.tile_pool(name="psB", bufs=4, space="PSUM") as psB,
        ):
            # ---------- constants ----------
            ident = consts.tile([P, P], bf16)
            make_identity(nc, ident)
            eps_t = consts.tile([P, 1], f32)
            nc.vector.memset(eps_t, EPS)
            ones1 = consts.tile([P, P], bf16)  # row of ones at partition 64
            nc.vector.memset(ones1[64:65, :], 1.0)
            bq_sb, bk_sb = {}, {}
            for pre in ("sa", "ca"):
                bq_sb[pre] = consts.tile([P, dtt], f32, tag=f"bq{pre}",
                                         name=f"bq{pre}")
                nc.sync.dma_start(out=bq_sb[pre], in_=bq_d[pre][:])
                bk_sb[pre] = consts.tile([P, dtt], f32, tag=f"bk{pre}",
                                         name=f"bk{pre}")
                nc.sync.dma_start(out=bk_sb[pre], in_=bk_d[pre][:])
            b1_sb = consts.tile([P, ft], f32, tag="b1")
            nc.sync.dma_start(out=b1_sb, in_=b1_d[:])

            def opt_bcast(dram, flag, tag):
                if not flag:
                    return None
                sb = consts.tile([P, d], f32, tag=tag, name=tag)
                nc.sync.dma_start(out=sb, in_=dram[:])
                return sb

            bv_sb = {pre: opt_bcast(bv_d[pre], flags[f"bias_v_{pre}"],
                                    f"bv{pre}") for pre in ("sa", "ca")}
            bo_sb = {pre: opt_bcast(bo_d[pre], flags[f"bias_o_{pre}"],
                                    f"bo{pre}") for pre in ("sa", "ca")}
            b2_sb = opt_bcast(b2_d, flags["bias_2"], "b2")
            lng_sb = [opt_bcast(lng_d[i], flags[f"ln_aff{i + 1}"], f"lg{i}")
                      for i in range(3)]
            lnb_sb = [opt_bcast(lnb_d[i], flags[f"ln_aff{i + 1}"], f"lb{i}")
                      for i in range(3)]

            mca_sb = None
            if flags["mask_ca"]:
                mca_sb = consts.tile([P, tt], f32, tag="mca")
                nc.sync.dma_start(out=mca_sb, in_=mcacol_d[:])

            maskT_sb = None
            if flags["mask_sa"]:
                maskT_sb = bigp.tile([P, tt, tq], f32, tag="big",
                                     name="maskT_sb")
                nc.sync.dma_start(
                    out=maskT_sb,
                    in_=maskT_d[:].rearrange("(k p) q -> p k q", p=P))

            # residual stream: own tq rows, fp32 token-major
            resid = residp.tile([P, qtt, d], f32)
            nc.sync.dma_start(
                out=resid, in_=x_d[0:tq, :].rearrange("(q p) d -> p q d", p=P))

            ctxT = ctxp.tile([P, dtt, t], bf16, tag="ctxT")
            nc.sync.dma_start(
                out=ctxT, in_=ctxT_d[:].rearrange("(k p) t -> p k t", p=P))

            # ---------- helpers ----------
            def load_w(dram, name):
                w = wmatp.tile([P, dtt, d], bf16, tag="wmat", name=name)
                nc.sync.dma_start(
                    out=w, in_=dram[:].rearrange("(k p) n -> p k n", p=P))
                return w

            def layernorm_to_T(src_ap_fn, ntiles, dst_T, iln):
                """src_ap_fn(i) -> ([P, d] f32 AP, is_dram). Writes bf16
                PE-transposed into dst_T[:, :, i*P:(i+1)*P]."""
                g_sb, b_sb = lng_sb[iln], lnb_sb[iln]
                for i in range(ntiles):
                    src, is_dram = src_ap_fn(i)
                    if is_dram:
                        xt = work.tile([P, d], f32, tag="sm4")
                        nc.sync.dma_start(out=xt, in_=src)
                        src = xt
                    st = work.tile([P, bnsub, 6], f32, tag="lnst")
                    sub = d // bnsub
                    for s in range(bnsub):
                        nc.vector.bn_stats(
                            out=st[:, s, :],
                            in_=src[:, s * sub:(s + 1) * sub])
                    mv = work.tile([P, 2], f32, tag="lnmv")
                    nc.vector.bn_aggr(out=mv, in_=st)
                    rstd = work.tile([P, 1], f32, tag="lnrs")
                    nc.scalar.activation(out=rstd, in_=mv[:, 1:2],
                                         func=AF.Sqrt, bias=eps_t, scale=1.0)
                    nc.vector.reciprocal(out=rstd, in_=rstd)
                    nmr = work.tile([P, 1], f32, tag="lnnm")
                    nc.vector.tensor_mul(out=nmr, in0=mv[:, 0:1], in1=rstd)
                    nc.vector.tensor_scalar_mul(out=nmr, in0=nmr, scalar1=-1.0)
                    hb = work.tile([P, d], bf16, tag="lnh")
                    if g_sb is None:
                        nc.vector.tensor_scalar(
                            out=hb, in0=src, scalar1=rstd, scalar2=nmr,
                            op0=ALU.mult, op1=ALU.add)
                    else:
                        hf = work.tile([P, d], f32, tag="lnhf")
                        nc.vector.tensor_scalar(
                            out=hf, in0=src, scalar1=rstd, scalar2=nmr,
                            op0=ALU.mult, op1=ALU.add)
                        nc.vector.tensor_mul(out=hf, in0=hf, in1=g_sb)
                        nc.vector.tensor_add(out=hb, in0=hf, in1=b_sb)
                    for dt_i in range(dtt):
                        tp = psB.tile([P, P], bf16, tag="psB", name="tp")
                        nc.tensor.transpose(tp,
                                            hb[:, dt_i * P:(dt_i + 1) * P],
                                            ident)
                        nc.vector.tensor_copy(
                            out=dst_T[:, dt_i, i * P:(i + 1) * P],
                            in_=tp)

            def proj_fm(dst_T, srcT, w_sb, ncols, b_sb, scale):
                """Feature-major projection: dst_T [P, dtt, ncols] bf16."""
                nchunks = max(1, ncols // NCHUNK)
                cs = min(NCHUNK, ncols)
                for m in range(dtt):
                    for ch in range(nchunks):
                        ps = psB.tile([P, cs], f32, tag="psB", name="pj")
                        for k in range(dtt):
                            nc.tensor.matmul(
                                ps, w_sb[:, k, m * P:(m + 1) * P],
                                srcT[:, k, ch * cs:(ch + 1) * cs],
                                start=(k == 0), stop=(k == dtt - 1))
                        nc.vector.tensor_scalar(
                            out=dst_T[:, m, ch * cs:(ch + 1) * cs], in0=ps,
                            scalar1=scale, scalar2=b_sb[:, m:m + 1],
                            op0=ALU.mult, op1=ALU.add)

            def proj_vaug(dst, srcT, w_sb, b_bcast):
                """Token-major V with per-head ones column:
                dst [P, tt, h*65] bf16; head hh at cols hh*65..hh*65+64,
                col hh*65+64 == 1.0 (softmax denominator trick)."""
                for ti in range(tt):
                    for ch in range(ncd):
                        ps = psB.tile([P, csd], f32, tag="psB", name="pv")
                        for k in range(dtt):
                            nc.tensor.matmul(
                                ps, srcT[:, k, ti * P:(ti + 1) * P],
                                w_sb[:, k, ch * csd:(ch + 1) * csd],
                                start=(k == 0), stop=(k == dtt - 1))
                        hperc = csd // dh    # heads per chunk
                        for hj in range(hperc):
                            hh = ch * hperc + hj
                            o = dst[:, ti, hh * (dh + 1):hh * (dh + 1) + dh]
                            if b_bcast is None:
                                nc.vector.tensor_copy(
                                    out=o, in_=ps[:, hj * dh:(hj + 1) * dh])
                            else:
                                nc.vector.tensor_add(
                                    out=o, in0=ps[:, hj * dh:(hj + 1) * dh],
                                    in1=b_bcast[:, ch * csd + hj * dh:
                                                ch * csd + (hj + 1) * dh])
                    ones_ap = dst[:, ti, :].rearrange(
                        "p (hh c) -> p hh c", c=dh + 1)[:, :, dh:dh + 1]
                    nc.vector.memset(ones_ap, 1.0)

            def attention(qT, kT, vaug, use_maskT, use_mca, wo_sb, bo_bcast,
                          do_cov, covT_sb):
                OTn = otnp.tile([P, dtt, tq], bf16, tag="OTn", name="OTn")
                if do_cov:
                    nc.vector.memset(covT_sb, 0.0)
                for hh in range(h):
                    pr, sub = hh // 2, hh % 2
                    lo = 64 * sub
                    # ST scores + exp, per k-tile
                    est = work.tile([P, tt, tq], bf16, tag="est", name="est")
                    for kt in range(tt):
                        sp = psB.tile([P, tq], f32, tag="psB", name="sps")
                        nc.tensor.matmul(
                            sp[:, 0:tq],
                            kT[lo:lo + dh, pr, kt * P:(kt + 1) * P],
                            qT[lo:lo + dh, pr, :],
                            start=True, stop=True, tile_position=(lo, 0))
                        if use_maskT:
                            sm = work.tile([P, tq], f32, tag="sm4",
                                           name="sm")
                            nc.vector.tensor_add(out=sm, in0=sp,
                                                 in1=maskT_sb[:, kt, :])
                            nc.scalar.activation(out=est[:, kt, :], in_=sm,
                                                 func=AF.Exp, bias=0.0,
                                                 scale=1.0)
                        elif use_mca:
                            nc.scalar.activation(
                                out=est[:, kt, :], in_=sp, func=AF.Exp,
                                bias=mca_sb[:, kt:kt + 1], scale=1.0)
                        else:
                            nc.scalar.activation(out=est[:, kt, :], in_=sp,
                                                 func=AF.Exp, bias=0.0,
                                                 scale=1.0)
                    # AV with fused denominator (row 64)
                    av = psB.tile([P, tq], f32, tag="psB", name="av")
                    for kt in range(tt):
                        nc.tensor.matmul(
                            av[0:dh + 1, :],
                            vaug[:, kt, hh * (dh + 1):(hh + 1) * (dh + 1)],
                            est[:, kt, :],
                            start=(kt == 0), stop=(kt == tt - 1))
                    # reciprocal of denominators (partition 64), broadcast
                    # to all partitions via K=1 ones-matmul
                    rb_sb = work.tile([P, tq], bf16, tag="rb", name="rb")
                    with nc.allow_low_precision(
                            reason="softmax recip row in bf16, matches "
                                   "bf16 probability envelope"):
                        nc.vector.reciprocal(out=rb_sb[64:65, :],
                                             in_=av[64:65, :])
                    rb_ps = psB.tile([P, tq], f32, tag="psB", name="rbps")
                    nc.tensor.matmul(rb_ps, ones1[64:65, :],
                                     rb_sb[64:65, :], start=True,
                                     stop=True, tile_position=(64, 0))
                    nc.vector.tensor_copy(out=rb_sb, in_=rb_ps)
                    # normalized output rows for this head (feature-major).
                    # DVE cannot shift partitions, so odd heads (rows 64:128
                    # of OTn) go through a small partition-shifting DMA.
                    if lo == 0:
                        nc.vector.tensor_mul(
                            out=OTn[0:dh, pr, :], in0=av[0:dh, :],
                            in1=rb_sb[0:dh, :])
                    else:
                        otmp = work.tile([P, tq], bf16, tag="otmp",
                                         name="otmp")
                        nc.vector.tensor_mul(
                            out=otmp[0:dh, :], in0=av[0:dh, :],
                            in1=rb_sb[0:dh, :])
                        nc.sync.dma_start(out=OTn[lo:lo + dh, pr, :],
                                          in_=otmp[0:dh, :])
                    if do_cov:
                        # covT += est * recip  (fp32, per k-tile)
                        for kt in range(tt):
                            pt = work.tile([P, tq], f32, tag="sm4",
                                           name="pt")
                            nc.vector.tensor_mul(out=pt, in0=est[:, kt, :],
                                                 in1=rb_sb)
                            nc.vector.tensor_add(out=covT_sb[:, kt, :],
                                                 in0=covT_sb[:, kt, :],
                                                 in1=pt)
                # output projection + residual, per query tile
                for qt in range(qtt):
                    op = psA.tile([P, d], f32, tag="psA", name="op")
                    for ch in range(ncd):
                        for k in range(dtt):
                            nc.tensor.matmul(
                                op[:, ch * csd:(ch + 1) * csd],
                                OTn[:, k, qt * P:(qt + 1) * P],
                                wo_sb[:, k, ch * csd:(ch + 1) * csd],
                                start=(k == 0), stop=(k == dtt - 1))
                    if bo_bcast is not None:
                        nc.vector.tensor_add(out=op, in0=op, in1=bo_bcast)
                    nc.vector.tensor_add(out=resid[:, qt, :],
                                         in0=resid[:, qt, :], in1=op)
                if do_cov:
                    for kt in range(tt):
                        cs_t = work.tile([P, tq], f32, tag="sm4", name="cs")
                        nc.vector.tensor_scalar_mul(out=cs_t,
                                                    in0=covT_sb[:, kt, :],
                                                    scalar1=1.0 / h)
                        nc.sync.dma_start(
                            out=covT_d[kt * P:(kt + 1) * P, :], in_=cs_t)

            # ---------- phase 1: LN1 over full sequence -> h1T ----------
            h1T = hTp.tile([P, dtt, t], bf16, tag="hT", name="h1T")
            layernorm_to_T(
                lambda i: (x_d[i * P:(i + 1) * P, :], True), tt, h1T, 0)

            # ---------- phase 2: self-attention ----------
            wq = load_w(w_d["sa_q"], "wqs")
            qT = kvp.tile([P, dtt, tq], bf16, tag="qT", name="qT1")
            proj_fm(qT, h1T[:, :, 0:tq], wq, tq, bq_sb["sa"], dh ** -0.5)
            wk = load_w(w_d["sa_k"], "wks")
            kT = kvp.tile([P, dtt, t], bf16, tag="kT", name="kT1")
            proj_fm(kT, h1T, wk, t, bk_sb["sa"], 1.0)
            wv = load_w(w_d["sa_v"], "wvs")
            vaug = kvp.tile([P, tt, h * (dh + 1)], bf16, tag="vaug",
                            name="va1")
            proj_vaug(vaug, h1T, wv, bv_sb["sa"])
            wo = load_w(w_d["sa_o"], "wos")
            attention(qT, kT, vaug, flags["mask_sa"], False, wo, bo_sb["sa"],
                      False, None)

            # ---------- phase 3: LN2 -> h2T, cross-attention ----------
            h2T = hTp.tile([P, dtt, tq], bf16, tag="hT", name="h2T")
            layernorm_to_T(lambda i: (resid[:, i, :], False), qtt, h2T, 1)
            wq = load_w(w_d["ca_q"], "wqc")
            qT2 = kvp.tile([P, dtt, tq], bf16, tag="qT", name="qT2")
            proj_fm(qT2, h2T, wq, tq, bq_sb["ca"], dh ** -0.5)
            wk = load_w(w_d["ca_k"], "wkc")
            kT2 = kvp.tile([P, dtt, t], bf16, tag="kT", name="kT2")
            proj_fm(kT2, ctxT, wk, t, bk_sb["ca"], 1.0)
            wv = load_w(w_d["ca_v"], "wvc")
            vaug2 = kvp.tile([P, tt, h * (dh + 1)], bf16, tag="vaug",
                             name="va2")
            proj_vaug(vaug2, ctxT, wv, bv_sb["ca"])
            wo = load_w(w_d["ca_o"], "woc")
            covT_sb = bigp.tile([P, tt, tq], f32, tag="big", name="covT_sb")
            attention(qT2, kT2, vaug2, False, flags["mask_ca"], wo,
                      bo_sb["ca"], True, covT_sb)

            # ---------- phase 4: LN3 -> h3T, FFN ----------
            h3T = hTp.tile([P, dtt, tq], bf16, tag="hT", name="h3T")
            layernorm_to_T(lambda i: (resid[:, i, :], False), qtt, h3T, 2)
            fT = bigp.tile([P, ft, tq], bf16, tag="big", name="fT")
            for fc in range(f // csf):
                w1c = wmatp.tile([P, dtt, csf], bf16, tag="wmat", name="w1c")
                nc.sync.dma_start(
                    out=w1c,
                    in_=w1_d[:, fc * csf:(fc + 1) * csf].rearrange(
                        "(k p) n -> p k n", p=P))
                for fm in range(csf // P):
                    fi = fc * (csf // P) + fm
                    ps = psB.tile([P, tq], f32, tag="psB", name="pf")
                    for k in range(dtt):
                        nc.tensor.matmul(
                            ps, w1c[:, k, fm * P:(fm + 1) * P],
                            h3T[:, k, :],
                            start=(k == 0), stop=(k == dtt - 1))
                    nc.scalar.activation(out=fT[:, fi, :], in_=ps,
                                         func=AF.Relu,
                                         bias=b1_sb[:, fi:fi + 1], scale=1.0)
            # FF2: stream W2 in dtt-row chunks; two qt outputs per sweep
            kchunks = ft // dtt
            for half in range((qtt + 1) // 2):
                qts = [q for q in (2 * half, 2 * half + 1) if q < qtt]
                ops = [psA.tile([P, d], f32, tag="psA", name=f"op{qi}")
                       for qi in range(len(qts))]
                for kc in range(kchunks):
                    w2c = wmatp.tile([P, dtt, d], bf16, tag="wmat",
                                     name="w2c")
                    nc.sync.dma_start(
                        out=w2c,
                        in_=w2_d[kc * dtt * P:(kc + 1) * dtt * P, :].rearrange(
                            "(k p) n -> p k n", p=P))
                    for qi, qt in enumerate(qts):
                        for ch in range(ncd):
                            for k in range(dtt):
                                kk = kc * dtt + k
                                nc.tensor.matmul(
                                    ops[qi][:, ch * csd:(ch + 1) * csd],
                                    fT[:, kk, qt * P:(qt + 1) * P],
                                    w2c[:, k, ch * csd:(ch + 1) * csd],
                                    start=(kk == 0), stop=(kk == ft - 1))
                for qi, qt in enumerate(qts):
                    if b2_sb is not None:
                        nc.vector.tensor_add(out=ops[qi], in0=ops[qi],
                                             in1=b2_sb)
                    nc.vector.tensor_add(out=resid[:, qt, :],
                                         in0=resid[:, qt, :], in1=ops[qi])
                    nc.sync.dma_start(out=xout_d[qt * P:(qt + 1) * P, :],
                                      in_=resid[:, qt, :])
    nc.finalize()   # Bacc legalization (reg alloc, event-semaphore splits)
    return nc


def _prep_inputs(x, context, mask_tgt, mask_src, weights, t, tq, d, h, f):
    """Build per-core in_maps. Returns (in_maps, flags, perms)."""
    ft, dtt, tt = f // P, d // P, t // P
    b = x.shape[0]
    qsh = t // tq

    def bf(a):
        return np.ascontiguousarray(a.astype(ml_dtypes.bfloat16))

    def f32c(a):
        return np.ascontiguousarray(a.astype(np.float32))

    def pp(vec, ntiles, scale=1.0):
        return f32c((vec.astype(np.float32) * scale).reshape(ntiles, P).T)

    def bcast(vec):
        return f32c(np.broadcast_to(vec.astype(np.float32)[None, :], (P, d)))

    flags = {
        "mask_sa": bool(mask_tgt.any()),
        "mask_ca": bool(mask_src.any()),
        "bias_v_sa": bool(np.any(weights["sa_bv"])),
        "bias_v_ca": bool(np.any(weights["ca_bv"])),
        "bias_o_sa": bool(np.any(weights["sa_bo"])),
        "bias_o_ca": bool(np.any(weights["ca_bo"])),
        "bias_2": bool(np.any(weights["ff_b2"])),
        "ln_aff1": not (np.all(weights["ln1_g"] == 1)
                        and not np.any(weights["ln1_b"])),
        "ln_aff2": not (np.all(weights["ln2_g"] == 1)
                        and not np.any(weights["ln2_b"])),
        "ln_aff3": not (np.all(weights["ln3_g"] == 1)
                        and not np.any(weights["ln3_b"])),
    }

    shared = {
        "wq_sa": bf(weights["sa_Wq"]), "wk_sa": bf(weights["sa_Wk"]),
        "wv_sa": bf(weights["sa_Wv"]), "wo_sa": bf(weights["sa_Wo"]),
        "wq_ca": bf(weights["ca_Wq"]), "wk_ca": bf(weights["ca_Wk"]),
        "wv_ca": bf(weights["ca_Wv"]), "wo_ca": bf(weights["ca_Wo"]),
        "w1": bf(weights["ff_W1"]), "w2": bf(weights["ff_W2"]),
        "bq_sa": pp(weights["sa_bq"], dtt, (d // h) ** -0.5),
        "bq_ca": pp(weights["ca_bq"], dtt, (d // h) ** -0.5),
        "bk_sa": pp(weights["sa_bk"], dtt),
        "bk_ca": pp(weights["ca_bk"], dtt),
        "b1": pp(weights["ff_b1"], ft),
        "bv_sa": bcast(weights["sa_bv"]), "bv_ca": bcast(weights["ca_bv"]),
        "bo_sa": bcast(weights["sa_bo"]), "bo_ca": bcast(weights["ca_bo"]),
        "b2": bcast(weights["ff_b2"]),
    }
    for nm in ("ln1", "ln2", "ln3"):
        shared[f"{nm}_g"] = bcast(weights[f"{nm}_g"])
        shared[f"{nm}_b"] = bcast(weights[f"{nm}_b"])

    mt = np.broadcast_to(mask_tgt, (b, t, t))
    ms = np.broadcast_to(mask_src.reshape(b, -1), (b, t))

    in_maps, perms = [], []
    for c in range(b * qsh):
        bi, qh = c // qsh, c % qsh
        qs = qh * tq
        perm = np.concatenate([np.arange(qs, qs + tq),
                               np.arange(0, qs),
                               np.arange(qs + tq, t)]).astype(np.int64)
        perms.append(perm)
        # maskT[k, q]: keys in permuted order, queries = own rows
        m = (mt[bi][qs:qs + tq][:, perm]).astype(np.float32).T * NEGMASK
        # mask_ca as per-k column bias [P, tt] (k = kt*P + p)
        mca = (ms[bi].astype(np.float32) * NEGMASK).reshape(tt, P).T
        im = dict(shared)
        im["x"] = f32c(x[bi][perm])
        im["ctxT"] = bf(context[bi].T)
        im["maskT"] = f32c(m)
        im["mca_col"] = f32c(mca)
        in_maps.append(im)
    return in_maps, flags, perms


_CACHE = {}


def _get_program(key, t, tq, d, h, f, flags):
    ck = (key, tuple(sorted(flags.items())))
    if ck not in _CACHE:
        _CACHE[ck] = _build_program(t, tq, d, h, f, flags)
    return _CACHE[ck]


def kernel(x, context, mask_tgt, mask_src,
           ln1_g, ln1_b, ln2_g, ln2_b, ln3_g, ln3_b,
           sa_Wq, sa_bq, sa_Wk, sa_bk, sa_Wv, sa_bv, sa_Wo, sa_bo,
           ca_Wq, ca_bq, ca_Wk, ca_bk, ca_Wv, ca_bv, ca_Wo, ca_bo,
           ff_W1, ff_b1, ff_W2, ff_b2, _run=None):
    x = np.asarray(x, np.float32)
    context = np.asarray(context, np.float32)
    mask_tgt = np.asarray(mask_tgt, bool)
    mask_src = np.asarray(mask_src, bool)
    weights = {k: np.asarray(v) for k, v in dict(
        ln1_g=ln1_g, ln1_b=ln1_b, ln2_g=ln2_g, ln2_b=ln2_b,
        ln3_g=ln3_g, ln3_b=ln3_b,
        sa_Wq=sa_Wq, sa_bq=sa_bq, sa_Wk=sa_Wk, sa_bk=sa_bk,
        sa_Wv=sa_Wv, sa_bv=sa_bv, sa_Wo=sa_Wo, sa_bo=sa_bo,
        ca_Wq=ca_Wq, ca_bq=ca_bq, ca_Wk=ca_Wk, ca_bk=ca_bk,
        ca_Wv=ca_Wv, ca_bv=ca_bv, ca_Wo=ca_Wo, ca_bo=ca_bo,
        ff_W1=ff_W1, ff_b1=ff_b1, ff_W2=ff_W2, ff_b2=ff_b2).items()}

    t, tq, d, h, f = T, TQ, D, H, F
    in_maps, flags, perms = _prep_inputs(
        x, context, mask_tgt, mask_src, weights, t, tq, d, h, f)
    nc = _get_program("full", t, tq, d, h, f, flags)
    if _run is None:
        res = run_bass_kernel_spmd(nc, in_maps,
                                   core_ids=list(range(len(in_maps)))).results
    else:
        res = _run(nc, in_maps)

    b, qsh = x.shape[0], t // tq
    xout = np.empty((b, t, d), np.float32)
    cov = np.empty((b, t, t), np.float32)
    for c, r in enumerate(res):
        bi, qh = c // qsh, c % qsh
        qs = qh * tq
        xout[bi, qs:qs + tq] = r["xout"]
        cov[bi, qs:qs + tq] = r["covT"].T
    return xout, cov
